# revision 35
# baseline (speedup 1.0000x reference)
"""GAU (Gated Attention Unit, relu^2 attention) Trainium2 Bass kernel, FP8.

Problem shapes: x [4, 2048, 2048] f32; W_hidden [2048, 8192]; W_qk [2048, 128];
W_out [4096, 2048]; out = GAU(x) + x.

Sharding (8 cores): core = 2*batch + h, h in {0,1}.  Each pair of cores
handles one batch.  Phase 1 splits the v-half of the hidden dim (each core
computes v for its 2048 columns); the halves are exchanged with a pairwise
AllGather (DRAM-staged, chunked, hidden under the gate phase).  Everything
downstream is split by sequence position: each core computes the gate, the
attention rows, the gated output and the final projection ONLY for its own
contiguous 1024 rows, using the full 4096-wide hidden dim -- so there is no
end-of-kernel ReduceScatter, and the output needs no cross-core reduction.
The gate over the full hidden dim for half the rows costs the same flops as
half the hidden dim for all rows; the qk projection is computed once for all
positions (for k) plus once for own positions (for q, via the host-sliced
xTown input -- SPMD code cannot index by core id).

All large matmuls run in fp8 e4m3 with perf_mode=DoubleRow (K=256 per
instruction); the small attention-scores matmul (K=128) is plain fp8.
fp32 PSUM accumulation throughout.  The GAU branch is ~3e-3 of the output
norm (the residual dominates), so ~10% fp8 error on the branch is ~3e-4
end-to-end.

fp8 range management (TRN e4m3 max +-240; >240 converts to Inf, subnormal
floor 2^-9).  Host pre-scales W_hidden and W_qk by 32 and W_out by 64 so
their rms is ~1.  Carried scales, verified against the actual seed-0 data:

  xT fp8           = x^T                (max 5.4)
  v_fp8            = 32 v               (max 155)
  gate_fp8         = 2^-6 gate          (max 0.083; psum * 2^-11)
  qT/kT fp8        true scale           (max 4.8)
  at_fp8           = 2^12 attn          (max 7.4;  relu stage scale 2^6/seq)
  og_fp8 = (32*2^12 attn@v) * gate_fp8 = 2^11 og   (max 122)
  out psum = og_fp8^T (64 Wout) = 2^17 branch  ->  out = xres + psum * 2^-17
"""

import math
import numpy as np
import ml_dtypes
from contextlib import ExitStack

import concourse.bass as bass
import concourse.bacc as bacc
import concourse.mybir as mybir
import concourse.tile as tile
from concourse.bass_utils import run_bass_kernel_spmd

BF16 = mybir.dt.bfloat16
F32 = mybir.dt.float32
FP8 = mybir.dt.float8e4
DR = mybir.MatmulPerfMode.DoubleRow
AF = mybir.ActivationFunctionType
ALU = mybir.AluOpType
P = 128

SH = 32.0          # host scale on W_hidden / W_qk
SO = 64.0          # host scale on W_out
SA = 4096.0        # fp8 scale of the attention matrix
GS = 2.0 ** -6     # fp8 carried scale of gateT
SGM = GS / SH      # psum -> gate linear-part multiplier
POSC = 1.0 / (SH * SA * GS * SO)   # final psum descale (2^-17)


def build_gau_nc(seq=2048, dim=2048, hh=2048, n_cores=8,
                 IC=None, DC=None, with_bhv=True):
    """Build the SPMD program.  hh = per-core v-half width (H/2)."""
    e = P  # qk dim
    H = 2 * hh          # full hidden width
    own = seq // 2      # own sequence rows per core
    nd = dim // P       # d-tiles (contraction tiles for x)
    njt = seq // P      # seq tiles (j)
    IC = IC or min(512, seq)   # i-chunk (moving free dim)
    n_ic = seq // IC           # chunks of the full sequence
    n_oc = own // IC           # chunks of the own half
    nct = hh // P       # v-half c-tiles
    nctg = H // P       # full-hidden c-tiles (gate/og/wout)
    DC = DC or min(512, dim)
    n_dc = dim // DC
    n_it = IC // P
    JG = njt // 4       # j-tiles per AllGather op
    rst = math.sqrt(SA) / seq
    pairs = [[2 * g, 2 * g + 1] for g in range(n_cores // 2)]

    nc = bacc.Bacc("TRN2", target_bir_lowering=False, debug=False,
                   num_devices=n_cores)

    xT_d = nc.dram_tensor("xT", [dim, seq], FP8, kind="ExternalInput")
    xTo_d = nc.dram_tensor("xTo", [dim, own], FP8, kind="ExternalInput")
    whv_d = nc.dram_tensor("whv", [dim, hh], FP8, kind="ExternalInput")
    whg_d = nc.dram_tensor("whg", [dim, H], FP8, kind="ExternalInput")
    wqk_d = nc.dram_tensor("wqk", [P, nd * e], FP8, kind="ExternalInput")
    wout_d = nc.dram_tensor("wout", [H, dim], FP8, kind="ExternalInput")
    bqk_d = nc.dram_tensor("bqk", [e, 1], F32, kind="ExternalInput")
    gq_d = nc.dram_tensor("gq", [e, 1], F32, kind="ExternalInput")
    bq_d = nc.dram_tensor("bq", [e, 1], F32, kind="ExternalInput")
    gk_d = nc.dram_tensor("gk", [e, 1], F32, kind="ExternalInput")
    bk_d = nc.dram_tensor("bk", [e, 1], F32, kind="ExternalInput")
    bhv_d = nc.dram_tensor("bhv", [1, hh], BF16, kind="ExternalInput")
    bhgT_d = nc.dram_tensor("bhgT", [P, nctg], F32, kind="ExternalInput")
    xres_d = nc.dram_tensor("xres", [own, dim], F32, kind="ExternalInput")
    out_d = nc.dram_tensor("out", [own, dim], F32, kind="ExternalOutput")

    with TileCtx(nc) as tc, ExitStack() as st:
        constp = st.enter_context(tc.tile_pool(name="const", bufs=1))
        psump = st.enter_context(tc.tile_pool(name="psum", bufs=8, space="PSUM"))
        dramp = st.enter_context(tc.tile_pool(name="dram", bufs=1, space="DRAM"))
        mainp = st.enter_context(tc.tile_pool(name="main", bufs=1))

        # v-exchange staging: own v half -> vag_in; pairwise AllGather per
        # JG j-tiles; vago[o] rows = [core0's JG*P rows, core1's JG*P rows]
        vag_in = dramp.tile([seq, hh], FP8, tag="vag_in", name="vag_in")
        vago = [dramp.tile([2 * JG * P, hh], FP8, tag=f"vago{o}",
                           name=f"vago{o}") for o in range(njt // JG)]

        # ---- constants ----
        wqk_sb = constp.tile([P, nd, e], FP8, tag="wqk")
        nc.sync.dma_start(wqk_sb[:], wqk_d[:])
        bqk_sb = constp.tile([e, 1], F32, tag="bqk")
        nc.sync.dma_start(bqk_sb[:], bqk_d[:])
        gq_sb = constp.tile([e, 1], F32, tag="gq")
        nc.sync.dma_start(gq_sb[:], gq_d[:])
        bq_sb = constp.tile([e, 1], F32, tag="bq")
        nc.sync.dma_start(bq_sb[:], bq_d[:])
        gk_sb = constp.tile([e, 1], F32, tag="gk")
        nc.sync.dma_start(gk_sb[:], gk_d[:])
        bk_sb = constp.tile([e, 1], F32, tag="bk")
        nc.sync.dma_start(bk_sb[:], bk_d[:])
        bhgT_sb = constp.tile([P, nctg], F32, tag="bhgT")
        nc.sync.dma_start(bhgT_sb[:], bhgT_d[:])
        bhgT6_sb = constp.tile([P, nctg], F32, tag="bhgT6")
        nc.vector.tensor_scalar(bhgT6_sb[:], bhgT_sb[:], GS, None, ALU.mult)
        bhv_sb = constp.tile([1, hh], BF16, tag="bhv")
        nc.sync.dma_start(bhv_sb[:], bhv_d[:])
        ones_sb = constp.tile([1, P], BF16, tag="ones")
        nc.vector.memset(ones_sb[:], 1.0)

        # tiny AllGather to warm the collective stream (first CC op
        # otherwise pays a ~50us cold-start)
        warm_in = dramp.tile([1, 64], F32, tag="warm_in", name="warm_in")
        warm_out = dramp.tile([2, 64], F32, tag="warm_out", name="warm_out")
        warm_sb = constp.tile([1, 64], F32, tag="warm_sb")
        nc.vector.memset(warm_sb[:], 0.0)
        nc.gpsimd.dma_start(warm_in[:], warm_sb[:])
        nc.gpsimd.collective_compute("AllGather", ALU.bypass,
                                     replica_groups=pairs,
                                     ins=[warm_in.opt()],
                                     outs=[warm_out.opt()])

        # persistent activations
        qT_sb = mainp.tile([e, own], FP8, tag="qT", name="qT")
        kT_sb = mainp.tile([e, seq], FP8, tag="kT", name="kT")
        v_sb = mainp.tile([P, njt, H], FP8, tag="v", name="v")
        gt_sb = mainp.tile([P, nctg, own], FP8, tag="gt", name="gt")

        with tc.tile_pool(name="ph1", bufs=1) as ph1p, \
             tc.tile_pool(name="wstream", bufs=1) as wsp:
            # sync-queue load order is the startup critical path: xT's first
            # chunk (qk), xTown (qk-own, issued first below), the first wv
            # tile (gates the v phase), THEN the bulk of xT
            xT_sb = ph1p.tile([P, nd, seq], FP8, tag="xT", name="xT")
            for d in range(nd):
                nc.sync.dma_start(xT_sb[:, d, 0:IC],
                                  xT_d[d * P:(d + 1) * P, 0:IC])
            xTo_sb = ph1p.tile([P, nd, own], FP8, tag="xTo", name="xTo")
            for d in range(nd):
                nc.sync.dma_start(xTo_sb[:, d, :],
                                  xTo_d[d * P:(d + 1) * P, :])
            wv0 = wsp.tile([P, nd, IC], FP8, tag="wv", bufs=2, name="wv0")
            for d in range(nd):
                nc.sync.dma_start(wv0[:, d, :],
                                  whv_d[d * P:(d + 1) * P, 0:IC])
            for d in range(nd):
                nc.sync.dma_start(xT_sb[:, d, IC:seq],
                                  xT_d[d * P:(d + 1) * P, IC:seq])

            # ---- qk projection ----
            # full sequence for k; own rows (xTown) for q.  silu via
            # sigmoid (ACT) + rebuild of the linear part on the DVE.
            with tc.tile_pool(name="qkp", bufs=1) as qkp:
                def qk_proj(rhs_sb, n_chunks, out_sb, g_sb, b_sb):
                    for ic in range(n_chunks):
                        isl = slice(ic * IC, (ic + 1) * IC)
                        ps = psump.tile([P, IC], F32, tag="ps", name="ps")
                        for kk in range(0, nd, 2):
                            nc.tensor.matmul(ps[:], wqk_sb[:, kk:kk + 2, :],
                                             rhs_sb[:, kk:kk + 2, isl],
                                             start=(kk == 0),
                                             stop=(kk == nd - 2),
                                             perf_mode=DR)
                        sg = qkp.tile([P, IC], F32, tag="sg1", bufs=2,
                                      name="sg")
                        nc.scalar.activation(sg[:], ps[:], AF.Sigmoid,
                                             bias=bqk_sb[:], scale=1.0 / SH)
                        u = qkp.tile([P, IC], F32, tag="u1", bufs=1, name="u")
                        nc.vector.tensor_scalar(u[:], ps[:], 1.0 / SH,
                                                bqk_sb[:], ALU.mult, ALU.add)
                        qkf = qkp.tile([P, IC], F32, tag="qkf", bufs=1,
                                       name="qkf")
                        nc.vector.tensor_tensor(qkf[:], u[:], sg[:], ALU.mult)
                        nc.vector.tensor_scalar(out_sb[:, isl], qkf[:],
                                                g_sb[:], b_sb[:],
                                                ALU.mult, ALU.add)

                qk_proj(xTo_sb, n_oc, qT_sb, gq_sb, bq_sb)
                qk_proj(xT_sb, n_ic, kT_sb, gk_sb, bk_sb)

            # ---- hidden, v part -> DRAM (own half), then pairwise AG ----
            n_cc = hh // IC
            for cc in range(n_cc):
                csl = slice(cc * IC, (cc + 1) * IC)
                if cc == 0:
                    wv = wv0
                else:
                    wv = wsp.tile([P, nd, IC], FP8, tag="wv", bufs=2,
                                  name="wv")
                    for d in range(nd):
                        nc.sync.dma_start(wv[:, d, :],
                                          whv_d[d * P:(d + 1) * P, csl])
                for jt in range(njt):
                    ps = psump.tile([P, IC], F32, tag="ps", name="ps")
                    for kk in range(0, nd, 2):
                        nc.tensor.matmul(ps[:],
                                         xT_sb[:, kk:kk + 2, jt * P:(jt + 1) * P],
                                         wv[:, kk:kk + 2, :],
                                         start=(kk == 0),
                                         stop=(not with_bhv and kk == nd - 2),
                                         perf_mode=DR)
                    if with_bhv:
                        # bhv host-scaled by 32 to match the psum scale
                        nc.tensor.matmul(ps[:], ones_sb[:], bhv_sb[:, csl],
                                         start=False, stop=True,
                                         skip_group_check=True)
                    sg = wsp.tile([P, IC], F32, tag="sgv", bufs=2, name="sgv")
                    nc.scalar.activation(sg[:], ps[:], AF.Sigmoid,
                                         scale=1.0 / SH)
                    vst = wsp.tile([P, IC], FP8, tag="vst", bufs=2, name="vst")
                    nc.vector.tensor_tensor(vst[:], ps[:], sg[:], ALU.mult)
                    nc.gpsimd.dma_start(vag_in[jt * P:(jt + 1) * P, csl],
                                        vst[:])

            # ---- hidden, gate part + staggered v AllGather ----
            # The AG's CCE work shares the SDMA datapath with regular DMA
            # and starves the gate weight stream if all 4 ops fire at once.
            # Putting the wg loads on the gpsimd queue and issuing AG op o
            # after the (o+2)th group's load makes each AG wait (via the
            # wg ring's buffer-reuse dependency) for a gate group to finish:
            # one AG per ~15us compute group, each hidden in its shadow.
            # Readbacks go on sync (idle after the wv stream); both halves
            # are read back -- SPMD code cannot index by core id.
            def v_allgather(o):
                nc.gpsimd.collective_compute(
                    "AllGather", ALU.bypass, replica_groups=pairs,
                    ins=[vag_in[o * JG * P:(o + 1) * JG * P, :].opt()],
                    outs=[vago[o].opt()])
                for g in range(2):
                    for jl in range(JG):
                        nc.sync.dma_start(
                            v_sb[:, o * JG + jl, g * hh:(g + 1) * hh],
                            vago[o][g * JG * P + jl * P:
                                    g * JG * P + (jl + 1) * P, :])

            CG = 4
            n_ag = njt // JG
            ago = 0
            for cg in range(nctg // CG):
                wg = wsp.tile([P, nd, CG * P], FP8, tag="wg", bufs=2,
                              name="wg")
                for d in range(nd):
                    nc.gpsimd.dma_start(wg[:, d, :],
                                        whg_d[d * P:(d + 1) * P,
                                              cg * CG * P:(cg + 1) * CG * P])
                if cg >= 2 and ago < n_ag:
                    v_allgather(ago)
                    ago += 1
                for cl in range(CG):
                    ct = cg * CG + cl
                    for ic in range(n_oc):
                        isl = slice(ic * IC, (ic + 1) * IC)
                        ps = psump.tile([P, IC], F32, tag="ps", name="ps")
                        for kk in range(0, nd, 2):
                            nc.tensor.matmul(ps[:],
                                             wg[:, kk:kk + 2,
                                                cl * P:(cl + 1) * P],
                                             xTo_sb[:, kk:kk + 2, isl],
                                             start=(kk == 0),
                                             stop=(kk == nd - 2),
                                             perf_mode=DR)
                        sgg = wsp.tile([P, IC], F32, tag="sgg", bufs=2,
                                       name="sgg")
                        nc.scalar.activation(sgg[:], ps[:], AF.Sigmoid,
                                             bias=bhgT_sb[:, ct:ct + 1],
                                             scale=1.0 / SH)
                        ug = wsp.tile([P, IC], F32, tag="ug", bufs=2,
                                      name="ug")
                        nc.vector.tensor_scalar(ug[:], ps[:], SGM,
                                                bhgT6_sb[:, ct:ct + 1],
                                                ALU.mult, ALU.add)
                        nc.vector.tensor_tensor(gt_sb[:, ct, isl], ug[:],
                                                sgg[:], ALU.mult)
            while ago < n_ag:
                v_allgather(ago)
                ago += 1

        # ---- attention + output (own rows only, full hidden dim) ----
        with tc.tile_pool(name="ph2", bufs=1) as ph2p, \
             tc.tile_pool(name="wop", bufs=1) as wop:
            at_sb = ph2p.tile([P, njt, own], FP8, tag="at", name="at")
            og_sb = [ph2p.tile([P, nctg, IC], FP8, tag=f"og{i}",
                               name=f"og{i}") for i in range(n_oc)]
            # all scores first (cheap), then av/out per chunk
            for ic in range(n_oc):
                isl = slice(ic * IC, (ic + 1) * IC)
                for jt in range(njt):
                    ps = psump.tile([P, IC], F32, tag="ps", name="ps")
                    nc.tensor.matmul(ps[:], kT_sb[:, jt * P:(jt + 1) * P],
                                     qT_sb[:, isl], start=True, stop=True)
                    rstage = ph2p.tile([P, IC], F32, tag="rstage", bufs=4,
                                       name="rstage")
                    nc.scalar.activation(rstage[:], ps[:], AF.Relu, scale=rst)
                    nc.vector.tensor_tensor(at_sb[:, jt, isl], rstage[:],
                                            rstage[:], ALU.mult)
            # ogT[all c, chunk] = (v^T @ attnT) * gateT, both chunks first
            # (both og buffers stay live so the out-projection can then run
            # dc-outer across chunks, loading each Wout column-block ONCE)
            for ic in range(n_oc):
                isl = slice(ic * IC, (ic + 1) * IC)
                og = og_sb[ic]
                for ct in range(nctg):
                    ps = psump.tile([P, IC], F32, tag="ps", name="ps")
                    for kk in range(0, njt, 2):
                        nc.tensor.matmul(ps[:],
                                         v_sb[:, kk:kk + 2, ct * P:(ct + 1) * P],
                                         at_sb[:, kk:kk + 2, isl],
                                         start=(kk == 0), stop=(kk == njt - 2),
                                         perf_mode=DR)
                    nc.vector.tensor_tensor(og[:, ct, :], ps[:],
                                            gt_sb[:, ct, isl], ALU.mult)
            # final rows: out[own rows, :] = 2^-17 ogT^T Wout + xres
            for dc in range(n_dc):
                wo = wop.tile([P, nctg, DC], FP8, tag="wo", bufs=2,
                              name="wo")
                for ct in range(nctg):
                    nc.gpsimd.dma_start(wo[:, ct, :],
                                        wout_d[ct * P:(ct + 1) * P,
                                               dc * DC:(dc + 1) * DC])
                for ic in range(n_oc):
                    for it in range(n_it):
                        orow = ic * IC + it * P
                        xr = ph2p.tile([P, DC], F32, tag="xr", bufs=2,
                                       name="xr")
                        nc.sync.dma_start(xr[:],
                                          xres_d[orow:orow + P,
                                                 dc * DC:(dc + 1) * DC])
                        ps = psump.tile([P, DC], F32, tag="ps", name="ps")
                        for kk in range(0, nctg, 2):
                            nc.tensor.matmul(ps[:],
                                             og_sb[ic][:, kk:kk + 2,
                                                       it * P:(it + 1) * P],
                                             wo[:, kk:kk + 2, :],
                                             start=(kk == 0),
                                             stop=(kk == nctg - 2),
                                             perf_mode=DR)
                        po = ph2p.tile([P, DC], F32, tag="po", bufs=2,
                                       name="po")
                        nc.scalar.mul(po[:], ps[:], POSC)
                        fo = ph2p.tile([P, DC], F32, tag="fo", bufs=2,
                                       name="fo")
                        nc.vector.tensor_tensor(fo[:], xr[:], po[:], ALU.add)
                        nc.scalar.dma_start(out_d[orow:orow + P,
                                                  dc * DC:(dc + 1) * DC],
                                            fo[:])

    nc.compile()
    return nc


def TileCtx(nc):
    return tile.TileContext(nc)


def own_rows(seq, h, IC=None):
    """Rows owned by pair-member h: the contiguous h-th half."""
    return np.arange(h * (seq // 2), (h + 1) * (seq // 2))


def _to_fp8(a):
    return np.clip(a, -224.0, 224.0).astype(ml_dtypes.float8_e4m3)


def make_in_maps(x, W_hidden, b_hidden, W_qk, b_qk, gamma_q, beta_q,
                 gamma_k, beta_k, W_out, b_out, n_cores=8, IC=None):
    """Host-side sharding/layout prep.  Returns per-core input dicts."""
    B, seq, dim = x.shape
    H2 = W_hidden.shape[1]
    H = H2 // 2
    hh = H // 2  # per-core v-half width
    nctg = H // P
    in_maps = []
    xT_cache = {}
    whg8 = _to_fp8(W_hidden[:, H:] * SH)
    wout8 = _to_fp8(W_out * SO)
    wqk8 = _to_fp8(np.ascontiguousarray(
        np.concatenate(np.split(W_qk * SH, dim // P, axis=0), axis=1)))
    bhgT = np.ascontiguousarray(
        b_hidden[H:].reshape(nctg, P).T).astype(np.float32)
    for core in range(n_cores):
        b, h = core // 2, core % 2
        if b not in xT_cache:
            xT_cache[b] = _to_fp8(np.ascontiguousarray(x[b].T))
        rows = own_rows(seq, h, IC)
        xres = (x[b][rows].astype(np.float32)
                + b_out.astype(np.float32)[None, :])
        cs = slice(h * hh, (h + 1) * hh)
        in_maps.append({
            "xT": xT_cache[b],
            "xTo": np.ascontiguousarray(xT_cache[b][:, rows]),
            "whv": _to_fp8(W_hidden[:, cs] * SH),
            "whg": whg8,
            "wqk": wqk8,
            "wout": wout8,
            "bqk": b_qk.reshape(-1, 1).astype(np.float32),
            "gq": gamma_q.reshape(-1, 1).astype(np.float32),
            "bq": beta_q.reshape(-1, 1).astype(np.float32),
            "gk": gamma_k.reshape(-1, 1).astype(np.float32),
            "bk": beta_k.reshape(-1, 1).astype(np.float32),
            "bhv": (b_hidden[cs] * SH).reshape(1, -1).astype(
                ml_dtypes.bfloat16),
            "bhgT": bhgT,
            "xres": xres,
        })
    return in_maps


_NC_CACHE = {}


def _get_nc(seq, dim, hh, n_cores, with_bhv=True):
    key = (seq, dim, hh, n_cores, with_bhv)
    if key not in _NC_CACHE:
        _NC_CACHE[key] = build_gau_nc(seq=seq, dim=dim, hh=hh,
                                      n_cores=n_cores, with_bhv=with_bhv)
    return _NC_CACHE[key]


def kernel(x, W_hidden, b_hidden, W_qk, b_qk, gamma_q, beta_q, gamma_k,
           beta_k, W_out, b_out):
    x = np.asarray(x)
    B, seq, dim = x.shape
    hh = W_hidden.shape[1] // 4
    n_cores = 2 * B
    with_bhv = bool(np.any(np.asarray(b_hidden)[: 2 * hh] != 0))
    nc = _get_nc(seq, dim, hh, n_cores, with_bhv=with_bhv)
    in_maps = make_in_maps(x, np.asarray(W_hidden), np.asarray(b_hidden),
                           np.asarray(W_qk), np.asarray(b_qk),
                           np.asarray(gamma_q), np.asarray(beta_q),
                           np.asarray(gamma_k), np.asarray(beta_k),
                           np.asarray(W_out), np.asarray(b_out),
                           n_cores=n_cores)
    res = run_bass_kernel_spmd(nc, in_maps, core_ids=list(range(n_cores)))
    out = np.empty((B, seq, dim), np.float32)
    for b in range(B):
        for h in range(2):
            out[b, own_rows(seq, h)] = res.results[2 * b + h]["out"]
    return out
